# revision 2
# baseline (speedup 1.0000x reference)
"""CosineSimilarityLoss (histogram binning) Trainium2 kernel.

Full inputs [2048, 4096] f32 x5 -> scalar f32 loss = 1 - mean(cosine_sim).

Strategy: data-parallel over 8 cores (256 rows each = 2 tiles of 128).
Per 128-row tile and spectrum, build f = floor(mz*2000) (exact trunc via
int32 round-trip + is_gt correction), then split the 2000 bins between
two engines running in parallel:

- DVE bins [NA, 2000): one native fp16 scalar_tensor_tensor per bin:
  H[j] = sum_k I[k] * [f[k] == j]  (is_equal + mult, fp32 accum_out).
- ACT bins [0, NA), in 125-bin blocks: the Scalar engine accumulates
  R[t] = sum_k relu(u[k] - t) on the block-localized plane
  u = (z - 125*c) * [z < 125*(c+1)],  z = f + I  (so out-of-block items
  contribute exactly 0 and fp32 accumulation stays small), while the DVE
  supplies counts S[j] = sum_k [f[k] >= j] via single-src tensor_scalar.
  Then exactly  H[j] = (R[t] - R[t+1]) - S[j+1] + S[block_top].

Dot products / squared norms via accumulating ops, cosine tail on-chip,
final mean on host.
"""

import os
import sys

sys.path.insert(0, "/opt/trn_rl_repo")

import numpy as np

from concourse import bacc, mybir
from concourse.bass_utils import run_bass_kernel_spmd
from concourse.tile import TileContext

N_CORES = 8
B, P, T = 2048, 4096, 4096
ROWS_PER_CORE = B // N_CORES  # 256
NUM_BINS = 2000
BLK = 125  # ACT block size in bins
NA = int(os.environ.get("KERNEL_NA", "875"))  # ACT bins, multiple of BLK
S_MODE = os.environ.get("KERNEL_S_MODE", "f32")  # f32 | f16
EPS = 1e-8
F32 = mybir.dt.float32
F16 = mybir.dt.float16
I32 = mybir.dt.int32
ALU = mybir.AluOpType
ACTF = mybir.ActivationFunctionType

assert NA % BLK == 0 and 0 <= NA <= NUM_BINS
N_BLOCKS = NA // BLK

_NC_CACHE = {}


def _build_planes(nc, pool, stage_mz, stage_int, stage_mask):
    """From staged f32 mz/intensity (+optional mask), build the planes the
    histogram passes need: f16 (floor bins, fp16), i16 (intensity, fp16),
    z (f + I, f32), f32f (floor bins, f32). Returns (f16, i16, z)."""
    y = pool.tile([128, P], F32, tag="y")
    fi = pool.tile([128, P], F32, tag="fi")
    ii = pool.tile([128, P], I32, tag="ii")
    g = pool.tile([128, P], F32, tag="g")
    z = pool.tile([128, P], F32, tag="z")
    f16 = pool.tile([128, P], F16, tag="f16")
    i16 = pool.tile([128, P], F16, tag="i16")

    # y = mz * 2000 (single-rounded f32, matches reference)
    nc.vector.tensor_scalar_mul(y[:], stage_mz[:], 2000.0)
    # floor via int32 round-trip; correct for round-to-nearest: f = fi - [fi > y]
    nc.vector.tensor_copy(ii[:], y[:])
    nc.vector.tensor_copy(fi[:], ii[:])
    nc.vector.scalar_tensor_tensor(g[:], fi[:], 0.0, y[:], ALU.bypass, ALU.is_gt)
    nc.vector.scalar_tensor_tensor(fi[:], fi[:], 0.0, g[:], ALU.bypass, ALU.subtract)
    if stage_mask is not None:
        nc.vector.scalar_tensor_tensor(
            stage_int[:], stage_int[:], 0.0, stage_mask[:], ALU.bypass, ALU.mult
        )
    # z = f + I (f32; ACT path needs intensity packed above the integer bin)
    nc.vector.scalar_tensor_tensor(z[:], fi[:], 0.0, stage_int[:], ALU.bypass, ALU.add)
    nc.vector.tensor_copy(f16[:], fi[:])
    nc.vector.tensor_copy(i16[:], stage_int[:])
    return f16, i16, z


def _histogram(nc, pools, f16, i16, z, H):
    """Fill H [128, NUM_BINS] f32 with the intensity-weighted histogram."""
    scratch, small = pools
    scr16 = scratch.tile([128, P], F16, tag="scr16")
    # --- DVE bins [NA, 2000): H[j] = sum I * [f == j]
    for j in range(NA, NUM_BINS):
        nc.vector.scalar_tensor_tensor(
            scr16[:], f16[:], float(j), i16[:], ALU.is_equal, ALU.mult,
            accum_out=H[:, j : j + 1],
        )
    if NA == 0:
        return
    # --- ACT bins [0, NA) in blocks of BLK
    R = small.tile([128, N_BLOCKS * (BLK + 1)], F32, tag="R")
    S = small.tile([128, N_BLOCKS * BLK], F32, tag="S")
    if S_MODE == "f16":
        Sa = small.tile([128, N_BLOCKS * BLK], F32, tag="Sa")
    m = scratch.tile([128, P], F32, tag="m")
    u = scratch.tile([128, P], F32, tag="u")
    scrA = scratch.tile([128, P], F16, tag="scrA")
    for c in range(N_BLOCKS):
        lo, hi = float(BLK * c), float(BLK * (c + 1))
        # u = (z - lo) * [z < hi] : out-of-block items contribute exactly 0
        nc.vector.tensor_scalar(m[:], z[:], hi, None, ALU.is_lt)
        nc.vector.scalar_tensor_tensor(u[:], z[:], -lo, m[:], ALU.add, ALU.mult)
        rbase = c * (BLK + 1)
        for t in range(BLK + 1):
            nc.scalar.activation(
                scrA[:], u[:], ACTF.Relu, bias=-float(t),
                accum_out=R[:, rbase + t : rbase + t + 1],
            )
        sbase = c * BLK
        for t in range(1, BLK + 1):  # S at global bins lo+1 .. lo+BLK
            jv = float(BLK * c + t)
            si = sbase + t - 1
            if S_MODE == "f32":
                nc.vector.tensor_scalar(
                    scr16[:], f16[:], jv, None, ALU.is_ge,
                    accum_out=S[:, si : si + 1],
                )
            else:
                nc.vector.tensor_scalar(
                    scr16[:, : P // 2], f16[:, : P // 2], jv, None, ALU.is_ge,
                    accum_out=Sa[:, si : si + 1],
                )
                nc.vector.tensor_scalar(
                    scr16[:, P // 2 :], f16[:, P // 2 :], jv, None, ALU.is_ge,
                    accum_out=S[:, si : si + 1],
                )
        if S_MODE == "f16":
            nc.vector.scalar_tensor_tensor(
                S[:, sbase : sbase + BLK], S[:, sbase : sbase + BLK], 0.0,
                Sa[:, sbase : sbase + BLK], ALU.bypass, ALU.add,
            )
    # --- assembly: H[j] = (R[t] - R[t+1]) - S[j+1] + S[block_top]
    for c in range(N_BLOCKS):
        rbase, sbase, hbase = c * (BLK + 1), c * BLK, c * BLK
        hs = H[:, hbase : hbase + BLK]
        nc.vector.scalar_tensor_tensor(
            hs, R[:, rbase : rbase + BLK], 0.0,
            R[:, rbase + 1 : rbase + BLK + 1], ALU.bypass, ALU.subtract,
        )
        nc.vector.scalar_tensor_tensor(
            hs, hs, 0.0, S[:, sbase : sbase + BLK], ALU.bypass, ALU.subtract,
        )
        # + S[block_top] (per-partition scalar broadcast along free dim)
        nc.vector.tensor_scalar(
            hs, hs, S[:, sbase + BLK - 1 : sbase + BLK], None, ALU.add
        )


def build_nc():
    if "nc" in _NC_CACHE:
        return _NC_CACHE["nc"]
    nc = bacc.Bacc("TRN2", target_bir_lowering=False, debug=False, num_devices=N_CORES)
    d_pmz = nc.dram_tensor("pmz", [ROWS_PER_CORE, P], F32, kind="ExternalInput")
    d_pint = nc.dram_tensor("pint", [ROWS_PER_CORE, P], F32, kind="ExternalInput")
    d_tmz = nc.dram_tensor("tmz", [ROWS_PER_CORE, T], F32, kind="ExternalInput")
    d_tint = nc.dram_tensor("tint", [ROWS_PER_CORE, T], F32, kind="ExternalInput")
    d_tmask = nc.dram_tensor("tmask", [ROWS_PER_CORE, T], F32, kind="ExternalInput")
    d_cos = nc.dram_tensor("cos", [128, 2], F32, kind="ExternalOutput")

    n_tiles = ROWS_PER_CORE // 128  # 2

    with TileContext(nc) as tc:
        with (
            tc.tile_pool(name="io", bufs=2) as io,
            tc.tile_pool(name="plane", bufs=2) as plane,
            tc.tile_pool(name="scratch", bufs=2) as scratch,
            tc.tile_pool(name="hist", bufs=2) as hist,
            tc.tile_pool(name="red", bufs=1) as red,
        ):
            dot = red.tile([128, 2], F32, tag="dot")
            pn2 = red.tile([128, 2], F32, tag="pn2")
            tn2 = red.tile([128, 2], F32, tag="tn2")

            for t in range(n_tiles):
                rs = slice(128 * t, 128 * (t + 1))
                HP = hist.tile([128, NUM_BINS], F32, tag="HP")
                HT = hist.tile([128, NUM_BINS], F32, tag="HT")
                for (d_mz, d_int, d_mask, H) in (
                    (d_pmz, d_pint, None, HP),
                    (d_tmz, d_tint, d_tmask, HT),
                ):
                    smz = io.tile([128, P], F32, tag="smz")
                    sint = io.tile([128, P], F32, tag="sint")
                    nc.sync.dma_start(smz[:], d_mz[rs, :])
                    nc.sync.dma_start(sint[:], d_int[rs, :])
                    smask = None
                    if d_mask is not None:
                        smask = io.tile([128, P], F32, tag="smask")
                        nc.sync.dma_start(smask[:], d_mask[rs, :])
                    f16, i16, z = _build_planes(nc, plane, smz, sint, smask)
                    _histogram(nc, (scratch, red), f16, i16, z, H)
                # reductions for this tile
                hsc = scratch.tile([128, NUM_BINS], F16, tag="hsc")
                nc.vector.scalar_tensor_tensor(
                    hsc[:], HP[:], 0.0, HT[:], ALU.bypass, ALU.mult,
                    accum_out=dot[:, t : t + 1],
                )
                nc.scalar.activation(
                    hsc[:], HP[:], ACTF.Square, accum_out=pn2[:, t : t + 1]
                )
                nc.scalar.activation(
                    hsc[:], HT[:], ACTF.Square, accum_out=tn2[:, t : t + 1]
                )

            _build_tail(nc, red, dot, pn2, tn2, d_cos)
    nc.compile()
    _NC_CACHE["nc"] = nc
    return nc


def _build_tail(nc, red, dot, pn2, tn2, d_cos):
    # cosine tail on [128, 2]
    pn = red.tile([128, 2], F32, tag="pn")
    tn = red.tile([128, 2], F32, tag="tn")
    rp = red.tile([128, 2], F32, tag="rp")
    rt = red.tile([128, 2], F32, tag="rt")
    den = red.tile([128, 2], F32, tag="den")
    cosv = red.tile([128, 2], F32, tag="cosv")
    nc.scalar.activation(pn[:], pn2[:], ACTF.Sqrt)
    nc.scalar.activation(tn[:], tn2[:], ACTF.Sqrt)
    # rp = 1/(pn+eps), rt = 1/(tn+eps)
    nc.vector.tensor_scalar_add(rp[:], pn[:], EPS)
    nc.vector.reciprocal(rp[:], rp[:])
    nc.vector.tensor_scalar_add(rt[:], tn[:], EPS)
    nc.vector.reciprocal(rt[:], rt[:])
    # dot_normalized = dot * rp * rt
    nc.vector.scalar_tensor_tensor(dot[:], dot[:], 0.0, rp[:], ALU.bypass, ALU.mult)
    nc.vector.scalar_tensor_tensor(dot[:], dot[:], 0.0, rt[:], ALU.bypass, ALU.mult)
    # pn_norm = clamp(pn * rp, eps); tn_norm likewise
    nc.vector.scalar_tensor_tensor(pn[:], pn[:], 0.0, rp[:], ALU.bypass, ALU.mult)
    nc.vector.scalar_tensor_tensor(tn[:], tn[:], 0.0, rt[:], ALU.bypass, ALU.mult)
    nc.vector.tensor_scalar_max(pn[:], pn[:], EPS)
    nc.vector.tensor_scalar_max(tn[:], tn[:], EPS)
    nc.vector.scalar_tensor_tensor(den[:], pn[:], 0.0, tn[:], ALU.bypass, ALU.mult)
    nc.vector.reciprocal(den[:], den[:])
    nc.vector.scalar_tensor_tensor(cosv[:], dot[:], 0.0, den[:], ALU.bypass, ALU.mult)
    nc.sync.dma_start(d_cos[:], cosv[:])


def make_in_maps(np_inputs):
    in_maps = []
    for c in range(N_CORES):
        rs = slice(c * ROWS_PER_CORE, (c + 1) * ROWS_PER_CORE)
        in_maps.append(
            {
                "pmz": np.ascontiguousarray(np_inputs["pred_mz"][rs]),
                "pint": np.ascontiguousarray(np_inputs["pred_intensity"][rs]),
                "tmz": np.ascontiguousarray(np_inputs["target_mz"][rs]),
                "tint": np.ascontiguousarray(np_inputs["target_intensity"][rs]),
                "tmask": np.ascontiguousarray(np_inputs["target_mask"][rs]),
            }
        )
    return in_maps


def kernel(pred_mz, pred_intensity, target_mz, target_intensity, target_mask):
    nc = build_nc()
    in_maps = make_in_maps(
        {
            "pred_mz": np.ascontiguousarray(pred_mz, dtype=np.float32),
            "pred_intensity": np.ascontiguousarray(pred_intensity, dtype=np.float32),
            "target_mz": np.ascontiguousarray(target_mz, dtype=np.float32),
            "target_intensity": np.ascontiguousarray(target_intensity, dtype=np.float32),
            "target_mask": np.ascontiguousarray(target_mask, dtype=np.float32),
        }
    )
    res = run_bass_kernel_spmd(nc, in_maps, core_ids=list(range(N_CORES)))
    cos = np.concatenate(
        [r["cos"].T.reshape(-1) for r in res.results]
    )  # [2048] rows in order: core-major, tile, partition
    mean = np.mean(cos.astype(np.float64))
    return np.float32(1.0 - mean)
